# revision 1
# baseline (speedup 1.0000x reference)
"""Trainium2 Bass kernel for nn_BallModel: 10M-step ballistic trajectory.

The reference recurrence (pos += vel*dt; vel += g*dt, recording pos) has the
closed form
    pos_i = pos0 + i*dt*vel0 + g*dt^2 * i*(i-1)/2  =  A + B*i + C*i^2
with A = pos0, B = dt*vel0 - C, C = (g*dt)*dt/2 (per component; C_x = 0).

Output is [10_000_000, 2] f32 (~80 MB), interleaved x,y.  Each of the 8 cores
produces a contiguous 2.5M-element slice (10 MB) -> memory-bound at the
per-core HBM write bandwidth (~380 GB/s measured => ~27 us drain floor).

Per core the slice is computed in 39 chunks of [128 partitions x 512 cols]
(one PSUM bank each).  With element index e = core_base + c*65536 + p*512 +
ce, the pair index is i = q + j where q = q(core, c, p) = core*1.25e6 +
c*32768 + p*256 is per-partition and j = ce>>1, comp = ce&1 alternates x/y
along the columns:

    out[p, ce] = basex(q)*even + basey(q)*odd + s1(q)*j*odd + resid(ce)
    basex(q) = A_x + B_x*q
    basey(q) = A_y + B_y*q + C*q^2
    s1(q)    = B_y + 2*C*q
    resid(ce) = B_x*j on even cols, C*j^2 on odd cols

Everything is generated by ONE K=10 bf16 matmul per chunk (PE streaming
throughput is N columns/cycle regardless of K): per-partition values live in
the stationary operand lhsT, per-column patterns in the moving operand rhs.
Values wider than bf16's 8 mantissa bits are split into 2-3 bf16 rows
(hi/lo/lo2) whose products accumulate exactly in the fp32 PSUM accumulator,
so the result is fp32-faithful (~1e-7 rel of the f64 closed form).  Each
PSUM chunk is copied to SBUF (alternating scalar/vector engines to split the
~24 us of 1x-rate PSUM-read copy work) and written out as one contiguous
256 KB HWDGE DMA; per-chunk DMAs measured faster than any batched variant
(the drain pipeline stays smoother).

Structural notes (walrus allows 1 embedded sync wait per instruction;
Bacc.finalize's generate_event_semaphores spills the rest into standalone
event-sem instructions, which cost ~130 ns each on the issuing engine):
 - built on bacc.Bacc, NOT raw bass.Bass, so that legalization runs;
 - every chunk gets its own SBUF output tile (10 MB of SBUF) so copies
   carry no WAR waits on earlier output DMAs;
 - each copy engine has its own PSUM pool so bank-reuse WAR waits stay
   single-semaphore.
"""

import sys
import types

import ml_dtypes
import numpy as np

import concourse.bacc as bacc
import concourse.bass as bass
import concourse.mybir as mybir
from concourse.bass_utils import run_bass_kernel_spmd
from concourse.tile import TileContext

# ---- problem constants (hardcoded; kernel.py must be self-contained) ----
N_PAIRS = 10_000_000
ELEMS = 2 * N_PAIRS  # 20,000,000 interleaved f32 values
N_CORES = 8
CE = ELEMS // N_CORES  # 2,500,000 elements per core
P = 128  # partitions
COLS = 512  # one PSUM bank of f32
CHUNK = P * COLS  # 65,536 elements per matmul chunk
NCH = -(-CE // CHUNK)  # 39 chunks/core (last one partial)
K = 10  # matmul contraction rows
HEAD_CH = 4  # chunks whose lhsT loads via the small fast head DMA
LAST_ROWS = -(-(CE - (NCH - 1) * CHUNK) // COLS)  # useful rows of final chunk (19)

# fp32-rounded constants, matching the reference's fp32 parameter rounding
DT = float(np.float32(0.01))
GDT_Y = float(np.float32(np.float32(-9.81) * np.float32(0.01)))  # fp32(g_y*dt)
C_Y = GDT_Y * DT / 2.0  # i^2 coefficient for y

_bf16 = ml_dtypes.bfloat16

# exposed for test.py introspection (exec_time_ns etc.)
LAST_RESULTS = None


def _ensure_axon_hooks_stub():
    """bass_utils imports antenv.axon_hooks when BASS_TRACE is set; some
    images lack that module.  Register a stub that degrades to the untraced
    path instead of crashing (test.py replaces it with a real NTFF hook)."""
    try:
        import antenv.axon_hooks  # noqa: F401

        return
    except ImportError:
        pass
    try:
        import antenv  # noqa: F401
    except ImportError:
        return
    stub = types.ModuleType("antenv.axon_hooks")
    stub.get_axon_ntff_profile_hook = lambda: None
    stub.set_axon_ntff_profile_hook = lambda h: None
    sys.modules["antenv.axon_hooks"] = stub


def _build_program() -> bass.Bass:
    # Bacc (not raw Bass): its finalize pipeline runs the sync-wait
    # legalization and register allocation walrus requires.
    nc = bacc.Bacc("TRN2", target_bir_lowering=False)
    # One small "head" input carries rh + the first HEAD_CH chunks' lhsT, so
    # a single fast DMA gates the first matmul; the lhsT tail loads
    # concurrently behind it.
    hd = nc.declare_dram_parameter(
        "hd", [K, COLS + HEAD_CH * P], mybir.dt.bfloat16, isOutput=False
    )
    lt_t = nc.declare_dram_parameter(
        "lt_t", [K, (NCH - HEAD_CH) * P], mybir.dt.bfloat16, isOutput=False
    )
    out = nc.declare_dram_parameter(
        "out", [NCH * P, COLS], mybir.dt.float32, isOutput=True
    )

    # Hybrid group schedule: the first RAMP_CH chunks ship as single-chunk
    # DMAs (minimum latency to the first output byte, ~0.7 us issue each);
    # the rest ship 4 chunks / 1 MB per DMA (descriptor generation is ~2.3x
    # cheaper per byte, so the sync issue stream stops being co-saturated
    # with the ~380 GB/s drain and finishes early).  The final chunk only
    # has LAST_ROWS useful rows and ships alone, trimmed.
    RAMP_CH = 8  # leading single-chunk DMAs (fast pipeline fill)
    TAIL_CH = 4  # trailing single-chunk DMAs (clean contiguous drain tail)
    groups = [[c] for c in range(RAMP_CH)]
    # the tiny partial final chunk ships early so the drain doesn't end on a
    # straggler completion
    groups.append([NCH - 1])
    c0 = RAMP_CH
    while c0 + 4 <= NCH - TAIL_CH:
        groups.append(list(range(c0, c0 + 4)))
        c0 += 4
    if c0 < NCH - TAIL_CH:
        groups.append(list(range(c0, NCH - TAIL_CH)))
    groups.extend([c] for c in range(NCH - TAIL_CH, NCH - 1))

    with TileContext(nc) as tc:
        with (
            tc.tile_pool(name="const", bufs=1) as cpool,
            tc.tile_pool(name="work", bufs=1) as wpool,
            tc.tile_pool(name="psum_a", bufs=2, space="PSUM") as ppool_a,
            tc.tile_pool(name="psum_b", bufs=2, space="PSUM") as ppool_b,
        ):
            hd_s = cpool.tile([K, COLS + HEAD_CH * P], mybir.dt.bfloat16)
            ltt_s = cpool.tile([K, (NCH - HEAD_CH) * P], mybir.dt.bfloat16)
            # Both on the sync HWDGE path.  The gpsimd SWDGE path stalls,
            # and issuing these from the scalar engine's HWDGE queue
            # hard-hangs the device.
            nc.sync.dma_start(hd_s[:], hd[:])
            nc.sync.dma_start(ltt_s[:], lt_t[:])
            rh_s = hd_s[:, :COLS]

            def lhsT(c):
                if c < HEAD_CH:
                    return hd_s[:, COLS + c * P : COLS + (c + 1) * P]
                c -= HEAD_CH
                return ltt_s[:, c * P : (c + 1) * P]

            for g, chunks in enumerate(groups):
                n = len(chunks)
                rows = LAST_ROWS if chunks[-1] == NCH - 1 else P
                # scalar-engine (ACT) copy share; singles alternate DVE/ACT
                na = n // 2 if n > 1 else (g % 2)
                nb = n - na
                pt_a = (
                    ppool_a.tile(
                        [P, na * COLS], mybir.dt.float32, name="pt_a", tag="pt_a"
                    )
                    if na
                    else None
                )
                pt_b = (
                    ppool_b.tile(
                        [P, nb * COLS], mybir.dt.float32, name="pt_b", tag="pt_b"
                    )
                    if nb
                    else None
                )
                for idx, cc in enumerate(chunks):
                    pt = pt_a if idx < na else pt_b
                    off = (idx if idx < na else idx - na) * COLS
                    nc.tensor.matmul(
                        pt[:, off : off + COLS], lhsT(cc), rh_s, start=True, stop=True
                    )
                ot = wpool.tile(
                    [P, n * COLS], mybir.dt.float32, name=f"ot{g}", tag=f"ot{g}"
                )
                if na:
                    nc.scalar.copy(ot[:rows, : na * COLS], pt_a[:rows, :])
                if nb:
                    nc.vector.tensor_copy(ot[:rows, na * COLS :], pt_b[:rows, :])
                dst = out[chunks[0] * P : chunks[-1] * P + rows, :]
                if n > 1:
                    dst = dst.rearrange("(j p) q -> p j q", p=P)
                    src = ot.rearrange("p (j q) -> p j q", q=COLS)
                else:
                    src = ot[:rows, :]
                nc.sync.dma_start(dst, src)
    nc.finalize()  # runs Bacc.compile(): reg alloc + sync-wait legalization
    return nc


def _split_bf16(x: np.ndarray, n: int):
    """Split x into n bf16 parts summing (nearly) exactly to x."""
    parts = []
    rem = np.asarray(x, dtype=np.float64).copy()
    for _ in range(n):
        p = rem.astype(_bf16)
        parts.append(p)
        rem = rem - p.astype(np.float64)
    return parts


def _host_tables(pos0: np.ndarray, vel0: np.ndarray):
    """Build per-core input tables (float64 math, cast at the end)."""
    ax, ay = float(pos0[0]), float(pos0[1])
    bx_c = DT * float(vel0[0])  # B_x (C_x = 0)
    by_c = DT * float(vel0[1]) - C_Y  # B_y

    # fixed rhs column patterns
    ce = np.arange(COLS)
    j = (ce >> 1).astype(np.float64)
    odd = (ce & 1).astype(np.float64)
    even = 1.0 - odd
    jodd = (j * odd).astype(_bf16)  # exact: j < 256
    resid = np.where(ce & 1 == 1, C_Y * j * j, bx_c * j)
    resid_hi, resid_lo = _split_bf16(resid, 2)
    rh_np = np.stack(
        [
            jodd,
            jodd,
            resid_hi,
            resid_lo,
            odd.astype(_bf16),
            odd.astype(_bf16),
            odd.astype(_bf16),
            even.astype(_bf16),
            even.astype(_bf16),
            even.astype(_bf16),
        ]
    )  # [K, COLS]

    in_maps = []
    c_idx = np.arange(NCH, dtype=np.float64)[:, None]  # [NCH, 1]
    p_idx = np.arange(P, dtype=np.float64)[None, :]  # [1, P]
    for k in range(N_CORES):
        q = k * (CE // 2) + c_idx * (CHUNK // 2) + p_idx * (COLS // 2)  # [NCH, P]
        s1_hi, s1_lo = _split_bf16(by_c + 2.0 * C_Y * q, 2)
        ones = np.ones_like(s1_hi)
        by3 = _split_bf16(ay + by_c * q + C_Y * q * q, 3)
        bx3 = _split_bf16(ax + bx_c * q, 3)
        rows = [s1_hi, s1_lo, ones, ones] + by3 + bx3
        lt_np = np.stack([r.reshape(-1) for r in rows])  # [K, NCH*P]
        in_maps.append(
            {
                "hd": np.ascontiguousarray(
                    np.concatenate([rh_np, lt_np[:, : HEAD_CH * P]], axis=1)
                ),
                "lt_t": np.ascontiguousarray(lt_np[:, HEAD_CH * P :]),
            }
        )
    return in_maps


def kernel(ball_mass, ball_initial_position, ball_initial_velocity) -> np.ndarray:
    global LAST_RESULTS
    pos0 = np.asarray(ball_initial_position, dtype=np.float32)
    vel0 = np.asarray(ball_initial_velocity, dtype=np.float32)

    _ensure_axon_hooks_stub()
    nc = _build_program()
    in_maps = _host_tables(pos0, vel0)
    res = run_bass_kernel_spmd(nc, in_maps, core_ids=list(range(N_CORES)))
    LAST_RESULTS = res

    parts = [
        np.asarray(r["out"], dtype=np.float32).reshape(-1)[:CE] for r in res.results
    ]
    return np.concatenate(parts).reshape(N_PAIRS, 2)


if __name__ == "__main__":
    import os

    pos0 = (
        np.load("/tmp/pos0.npy")
        if os.path.exists("/tmp/pos0.npy")
        else np.array([-1.866805, -0.25733662], np.float32)
    )
    vel0 = (
        np.load("/tmp/vel0.npy")
        if os.path.exists("/tmp/vel0.npy")
        else np.array([-0.847358, -1.5444987], np.float32)
    )
    outv = kernel(np.ones(()), pos0, vel0)
    i = np.arange(N_PAIRS, dtype=np.float64)[:, None]
    closed = (
        pos0.astype(np.float64)
        + i * DT * vel0.astype(np.float64)
        + np.array([0.0, GDT_Y * DT]) * i * (i - 1) / 2.0
    )
    err = np.abs(outv - closed)
    denom = np.maximum(np.abs(closed), 1e-12)
    print("closed-form maxabs-ratio rel err:", err.max() / np.abs(closed).max())
    print("closed-form max elementwise rel err:", (err / denom).max())

